# revision 13
# baseline (speedup 1.0000x reference)
"""Trainium2 Bass kernel for the sparse-attention CompiledTransformerLayer.

Math (derived from the reference):
  c0 = rowsum(mask0); attended = (mask0 @ x[:,:,0:16]) * r/(1-r), r = 1/(1+c0)
  out ch16:32 = attended @ W_o0.T
  out ch32    = c1 * W_o1[0,0], c1 = rowsum(mask1)
  out ch48:64 = a + b; 64:80 = a*b; 80:96 = (a > b), a = x ch0:16, b = ch16:32
  all other channels pass through from x.

Sharding: 8 cores = 4 batches x 2 query-halves (1024 queries each).

Tricks:
  - bool masks are DMA-transposed as uint16 byte-pairs (HWDGE xbar, 2-byte dtype),
    then fed to the PE matmul directly as float8e4: byte 0x01 is the fp8
    denormal 2^-9, so results are exactly scaled by 2^-9 (weights pre-scaled
    by 512 to compensate).
  - value weights are (x[:,:,0:16] @ W_o0.T) split hi+lo in bf16 for ~f32
    matmul precision; an extra ones*512 column yields c0 in the same psum.
  - rowsum(mask1) via an all-ones fp8 stationary matmul (exact).
"""
import sys
sys.path.insert(0, "/opt/trn_rl_repo")
import numpy as np
import ml_dtypes

import concourse.bass as bass
import concourse.mybir as mybir
from concourse import tile
from concourse.bass_utils import run_bass_kernel_spmd
from concourse.vector_clock import ScopedClock, VectorClock
from concourse.tile import add_dep_helper

B, S, D = 4, 2048, 128
QH = S // 2              # queries per core
NQ = 8                   # j2 blocks of 128 (each covers 256 keys)
DT = mybir.dt
AL = mybir.AluOpType

# walrus codegen rejects instructions with many sem waits; the Tile tail
# drain accumulates one wait per touched proc. Emit one single-wait drain
# per proc instead.
def _patched_dab(self, tick_clock, wait_clock):
    ticks = list(tick_clock.global_clock)
    for i, t in enumerate(ticks):
        if t <= 0:
            continue
        part = [t if j == i else 0 for j, t in enumerate(ticks)]
        d = self.nc.sync.drain()
        wait_clock.add_sem_waits(d.ins, ScopedClock({None: VectorClock(part)}))
    self.nc.sync.drain()
    self.nc.all_engine_barrier()
    popped = self.nc._tile_sem_poison_stack.pop()
    assert popped is self._sem_poison
    self.nc.clear_and_free_semaphores(list(self.sems.allocated().values()))
    self.nc.all_engine_barrier()
tile.TileContext._drain_and_barrier = _patched_dab


def _build_program():
    nc = bass.Bass()
    m0_d = nc.declare_dram_parameter("m0", [QH, S // 2], DT.uint16, isOutput=False)
    m1_d = nc.declare_dram_parameter("m1", [QH, S // 2], DT.uint16, isOutput=False)
    x_d = nc.declare_dram_parameter("xq", [QH, D], DT.float32, isOutput=False)
    whi_d = nc.declare_dram_parameter("whi", [128, NQ, 2, 17], DT.bfloat16, isOutput=False)
    wlo_d = nc.declare_dram_parameter("wlo", [128, NQ, 2, 17], DT.bfloat16, isOutput=False)
    wo1_d = nc.declare_dram_parameter("wo1", [128, 1], DT.float32, isOutput=False)
    out_d = nc.declare_dram_parameter("out", [QH, D], DT.float32, isOutput=True)

    x_view = None  # set below
    with tile.TileContext(nc) as tc, \
         tc.tile_pool(name="const", bufs=1) as cpool, \
         tc.tile_pool(name="masks", bufs=8) as mpool, \
         tc.tile_pool(name="work", bufs=2) as wpool, \
         tc.tile_pool(name="ps", bufs=1, space="PSUM") as ps:

        x_view = x_d[:].rearrange("(t p) c -> p t c", p=128)    # [128, 8, 128]
        o_view = out_d[:].rearrange("(t p) c -> p t c", p=128)

        # x loads first: zero-wait DMAs, and they precede every transpose so
        # the xbar-mode serialization never lands on them
        ots = []
        xdmas = []
        for h in range(2):
            ot = wpool.tile([128, 4, D], DT.float32, tag=f"ot{h}", name=f"ot{h}")
            xdmas.append(nc.sync.dma_start(ot[:], x_view[:, 4 * h:4 * (h + 1), :]))
            ots.append(ot)

        whi = cpool.tile([128, NQ, 2, 17], DT.bfloat16)
        wlo = cpool.tile([128, NQ, 2, 17], DT.bfloat16)
        nc.sync.dma_start(whi[:], whi_d[:])
        nc.sync.dma_start(wlo[:], wlo_d[:])
        wo1_raw = cpool.tile([128, 1], DT.float32)
        nc.sync.dma_start(wo1_raw[:], wo1_d[:])
        wo1 = cpool.tile([128, 1], DT.float32)
        nc.vector.tensor_copy(wo1[:], wo1_raw[:])   # absorb DMA wait off TT path
        ones8 = cpool.tile([128, 32], DT.float8e4)
        nc.vector.memset(ones8[:], 1.0)

        # psum accumulation groups per query-half
        S_ps = [ps.tile([32, 512], DT.float32, tag=f"S{h}", name=f"S{h}") for h in range(2)]
        C_ps = [ps.tile([32, 512], DT.float32, tag=f"C{h}", name=f"C{h}") for h in range(2)]

        # ---- matmul phase: stream mask tiles, accumulate ----
        for q in range(NQ):
            m0t = mpool.tile([128, QH], DT.uint16, tag="m0")
            nc.sync.dma_start(m0t[:], m0_d[:, 128 * q:128 * (q + 1)], transpose=True)
            m1t = mpool.tile([128, QH], DT.uint16, tag="m1")
            m1dma = nc.sync.dma_start(m1t[:], m1_d[:, 128 * q:128 * (q + 1)], transpose=True)
            m0r = m0t[:].bitcast(DT.float8e4).rearrange("p (i two) -> p i two", two=2)
            m1r = m1t[:].bitcast(DT.float8e4).rearrange("p (i two) -> p i two", two=2)
            for h in range(2):
                for par in range(2):
                    rhs0 = m0r[:, 512 * h:512 * (h + 1), par]
                    first = (q == 0 and par == 0)
                    last = (q == NQ - 1 and par == 1)
                    nc.tensor.matmul(S_ps[h][0:17, :], whi[:, q, par, :], rhs0,
                                     start=first, stop=False)
                    nc.tensor.matmul(S_ps[h][0:17, :], wlo[:, q, par, :], rhs0,
                                     start=False, stop=last)
                    rhs1 = m1r[:, 512 * h:512 * (h + 1), par]
                    last_mm = nc.tensor.matmul(C_ps[h][:], ones8[:], rhs1,
                                               start=first, stop=last)

        # x loads on Pool/SWDGE after all transpose DMAs (xbar-mode safety,
        # and they double as the xbar fence for the out-stores); only needed
        # in the post phase, so the delay hides under the matmul tail.

        # chain of tiny Pool DMAs, each absorbing exactly one sem for the
        # 1-wait-limited Pool out-stores: xbar serialization, then the two
        # x-load lanes
        xfence = cpool.tile([1, 3], DT.float32, name="xfence")
        f1 = nc.gpsimd.dma_start(xfence[0:1, 0:1], wo1_d[0:1, :])
        add_dep_helper(f1.ins, m1dma.ins, reason="xbar fence after last transpose")
        f2 = nc.gpsimd.dma_start(xfence[0:1, 1:2], wo1_d[0:1, :])
        add_dep_helper(f2.ins, xdmas[0].ins, reason="absorb x-load h0 lane")
        add_dep_helper(f2.ins, f1.ins, sync=False, reason="pool order")
        f3 = nc.gpsimd.dma_start(xfence[0:1, 2:3], wo1_d[0:1, :])
        add_dep_helper(f3.ins, xdmas[1].ins, reason="absorb x-load h1 lane")
        add_dep_helper(f3.ins, f2.ins, sync=False, reason="pool order")

        # ---- post phase per query-half ----
        for h in range(2):
            Ssb = wpool.tile([32, 512], DT.float32, tag="Ssb")
            nc.scalar.copy(Ssb[:], S_ps[h][:])
            Csb = wpool.tile([32, 512], DT.float32, tag="Csb")
            nc.scalar.copy(Csb[:], C_ps[h][:])

            TS = wpool.tile([32, 512], DT.float32, tag="TS")
            nc.vector.transpose(TS[:], Ssb[:])      # 16 in-place 32x32 blocks
            TC = wpool.tile([32, 512], DT.float32, tag="TC")
            nc.vector.transpose(TC[:], Csb[:])

            # att[128p, t, d] = S[d, 128t+p]; block (4t+m) of TS holds rows 32m..32m+32
            att = wpool.tile([128, 4, 32], DT.float32, tag="att")
            TSv = TS[:].rearrange("p (k d) -> p k d", d=32)     # [32, 16, 32]
            TCv = TC[:].rearrange("p (k d) -> p k d", d=32)
            for m in range(4):
                nc.vector.tensor_copy(att[32 * m:32 * m + 32, :, :], TSv[:, m::4, :])

            # scale chain on [128, 4]: c0 = att[:, :, 16]; w = r/(1-r), r=1/(1+c0)
            denom = wpool.tile([128, 4], DT.float32, tag="denom")
            nc.vector.tensor_scalar_add(denom[:], att[:, :, 16], 1.0)
            r_t = wpool.tile([128, 4], DT.float32, tag="r_t")
            nc.vector.reciprocal(r_t[:], denom[:])
            omr = wpool.tile([128, 4], DT.float32, tag="omr")
            nc.vector.tensor_scalar(omr[:], r_t[:], -1.0, 1.0, AL.mult, AL.add)
            nc.vector.tensor_scalar_max(omr[:], omr[:], 1e-9)
            romr = wpool.tile([128, 4], DT.float32, tag="romr")
            nc.vector.reciprocal(romr[:], omr[:])
            wcol = wpool.tile([128, 4], DT.float32, tag="wcol")
            nc.vector.tensor_tensor(wcol[:], r_t[:], romr[:], AL.mult)

            # output staging: x rows stream straight into the out tile
            ot = ots[h]
            lab = wpool.tile([1, 1], DT.float32, tag="lab", name=f"lab{h}")
            abs_cp = nc.vector.tensor_copy(lab[:], ot[0:1, 0, 0:1])

            atts = wpool.tile([128, 4, 16], DT.float32, tag="atts")
            for t in range(4):
                nc.vector.scalar_tensor_tensor(
                    atts[:, t, :], att[:, t, 0:16], wcol[:, t:t + 1],
                    att[:, t, 0:16], AL.mult, AL.bypass)

            # ch16:32 = attended
            cp1632 = nc.vector.tensor_copy(ot[:, :, 16:32], atts[:])
            add_dep_helper(cp1632.ins, abs_cp.ins, sync=False, reason="after lane absorb")
            # ch32 = c1 * W_o1 (gather TC blocks to full partitions first)
            c1col = wpool.tile([128, 4], DT.float32, tag="c1col")
            for m in range(4):
                nc.vector.tensor_copy(c1col[32 * m:32 * m + 32, :], TCv[:, m::4, 0])
            cstt = nc.vector.scalar_tensor_tensor(
                ot[:, :, 32:33].rearrange("p t one -> p (t one)"),
                c1col[:], wo1[:], c1col[:], AL.mult, AL.bypass)
            add_dep_helper(cstt.ins, abs_cp.ins, sync=False, reason="after lane absorb")
            # MLP: a = ch0:16, b = atts
            for alu, lo in ((AL.add, 48), (AL.mult, 64), (AL.is_lt, 80)):
                mlp = nc.vector.tensor_tensor(ot[:, :, lo:lo + 16], atts[:],
                                              ot[:, :, 0:16], alu)
                add_dep_helper(mlp.ins, abs_cp.ins, sync=False, reason="after lane absorb")

            nc.gpsimd.dma_start(o_view[:, 4 * h:4 * (h + 1), :], ot[:])

    return nc


_cached = {}


def kernel(x, mask0, mask1, W_o0, W_o1):
    x = np.asarray(x, dtype=np.float32)
    m0u8 = np.asarray(mask0).astype(np.uint8, copy=False)
    m1u8 = np.asarray(mask1).astype(np.uint8, copy=False)
    W_o0 = np.asarray(W_o0, dtype=np.float32)
    W_o1 = np.asarray(W_o1, dtype=np.float32)

    if "nc" not in _cached:
        _cached["nc"] = _build_program()
    nc = _cached["nc"]

    # u = values through the head-0 output projection; hi/lo split, x512
    in_maps = []
    for c in range(8):
        b, h = divmod(c, 2)
        u = x[b, :, 0:16] @ W_o0.T                      # (S, 16) f32
        u_hi = u.astype(ml_dtypes.bfloat16)
        u_lo = (u - u_hi.astype(np.float32)).astype(ml_dtypes.bfloat16)
        whi = np.zeros((128, NQ, 2, 17), dtype=ml_dtypes.bfloat16)
        wlo = np.zeros((128, NQ, 2, 17), dtype=ml_dtypes.bfloat16)
        for q in range(NQ):
            blk_hi = u_hi[256 * q:256 * (q + 1)]        # (256, 16)
            blk_lo = u_lo[256 * q:256 * (q + 1)]
            for par in range(2):
                whi[:, q, par, :16] = (blk_hi[par::2].astype(np.float32) * 512.0
                                       ).astype(ml_dtypes.bfloat16)
                wlo[:, q, par, :16] = (blk_lo[par::2].astype(np.float32) * 512.0
                                       ).astype(ml_dtypes.bfloat16)
            whi[:, q, :, 16] = 512.0                    # ones column -> c0 exactly
        sl = slice(QH * h, QH * (h + 1))
        in_maps.append({
            "m0": np.ascontiguousarray(m0u8[b, sl, :]).view(np.uint16),
            "m1": np.ascontiguousarray(m1u8[b, sl, :]).view(np.uint16),
            "xq": np.ascontiguousarray(x[b, sl, :]),
            "whi": whi,
            "wlo": wlo,
            "wo1": np.full((128, 1), 512.0 * float(W_o1[0, 0]), np.float32),
        })

    res = run_bass_kernel_spmd(nc, in_maps, list(range(8)))
    out = np.empty((B, S, D), np.float32)
    for c in range(8):
        b, h = divmod(c, 2)
        out[b, QH * h:QH * (h + 1), :] = res.results[c]["out"]
    return out


# revision 15
# speedup vs baseline: 11.6259x; 11.6259x over previous
"""Trainium2 Bass kernel for the sparse-attention CompiledTransformerLayer.

Math (derived from the reference):
  c0 = rowsum(mask0); attended = (mask0 @ x[:,:,0:16]) * r/(1-r), r = 1/(1+c0)
  out ch16:32 = attended @ W_o0.T
  out ch32    = c1 * W_o1[0,0], c1 = rowsum(mask1)
  out ch48:64 = a + b; 64:80 = a*b; 80:96 = (a > b), a = x ch0:16, b = ch16:32
  all other channels pass through from x.

Sharding: 8 cores = 4 batches x 2 query-halves (1024 queries each).

Tricks:
  - bool masks are DMA-transposed as uint16 byte-pairs (HWDGE xbar, 2-byte dtype),
    then fed to the PE matmul directly as float8e4: byte 0x01 is the fp8
    denormal 2^-9, so results are exactly scaled by 2^-9 (weights pre-scaled
    by 512 to compensate).
  - value weights are (x[:,:,0:16] @ W_o0.T) split hi+lo in bf16 for ~f32
    matmul precision; an extra ones*512 column yields c0 in the same psum.
  - rowsum(mask1) via an all-ones fp8 stationary matmul (exact).
"""
import sys
sys.path.insert(0, "/opt/trn_rl_repo")
import numpy as np
import ml_dtypes

import concourse.bass as bass
import concourse.mybir as mybir
from concourse import tile
from concourse.bass_utils import run_bass_kernel_spmd
from concourse.vector_clock import ScopedClock, VectorClock
from concourse.tile import add_dep_helper

B, S, D = 4, 2048, 128
QH = S // 2              # queries per core
NQ = 8                   # j2 blocks of 128 (each covers 256 keys)
DT = mybir.dt
AL = mybir.AluOpType

# walrus codegen rejects instructions with many sem waits; the Tile tail
# drain accumulates one wait per touched proc. Emit one single-wait drain
# per proc instead.
def _patched_dab(self, tick_clock, wait_clock):
    ticks = list(tick_clock.global_clock)
    for i, t in enumerate(ticks):
        if t <= 0:
            continue
        part = [t if j == i else 0 for j, t in enumerate(ticks)]
        d = self.nc.sync.drain()
        wait_clock.add_sem_waits(d.ins, ScopedClock({None: VectorClock(part)}))
    self.nc.sync.drain()
    self.nc.all_engine_barrier()
    popped = self.nc._tile_sem_poison_stack.pop()
    assert popped is self._sem_poison
    self.nc.clear_and_free_semaphores(list(self.sems.allocated().values()))
    self.nc.all_engine_barrier()
tile.TileContext._drain_and_barrier = _patched_dab


def _build_program():
    nc = bass.Bass()
    m0_d = nc.declare_dram_parameter("m0", [QH, S // 2], DT.uint16, isOutput=False)
    m1_d = nc.declare_dram_parameter("m1", [QH, S // 2], DT.uint16, isOutput=False)
    x_d = nc.declare_dram_parameter("xq", [QH, D], DT.float32, isOutput=False)
    whi_d = nc.declare_dram_parameter("whi", [128, NQ, 2, 17], DT.bfloat16, isOutput=False)
    wlo_d = nc.declare_dram_parameter("wlo", [128, NQ, 2, 17], DT.bfloat16, isOutput=False)
    wo1_d = nc.declare_dram_parameter("wo1", [128, 1], DT.float32, isOutput=False)
    out_d = nc.declare_dram_parameter("out", [QH, D], DT.float32, isOutput=True)

    x_view = None  # set below
    with tile.TileContext(nc) as tc, \
         tc.tile_pool(name="const", bufs=1) as cpool, \
         tc.tile_pool(name="masks", bufs=8) as mpool, \
         tc.tile_pool(name="work", bufs=2) as wpool, \
         tc.tile_pool(name="ps", bufs=1, space="PSUM") as ps:

        x_view = x_d[:].rearrange("(t p) c -> p t c", p=128)    # [128, 8, 128]
        o_view = out_d[:].rearrange("(t p) c -> p t c", p=128)

        # x loads first: zero-wait DMAs, and they precede every transpose so
        # the xbar-mode serialization never lands on them
        ots = []
        xdmas = []
        for h in range(2):
            ot = wpool.tile([128, 4, D], DT.float32, tag=f"ot{h}", name=f"ot{h}")
            xdmas.append(nc.sync.dma_start(ot[:], x_view[:, 4 * h:4 * (h + 1), :]))
            ots.append(ot)

        whi = cpool.tile([128, NQ, 2, 17], DT.bfloat16)
        wlo = cpool.tile([128, NQ, 2, 17], DT.bfloat16)
        nc.sync.dma_start(whi[:], whi_d[:])
        nc.sync.dma_start(wlo[:], wlo_d[:])
        wo1_raw = cpool.tile([128, 1], DT.float32)
        nc.sync.dma_start(wo1_raw[:], wo1_d[:])
        wo1 = cpool.tile([128, 1], DT.float32)
        nc.vector.tensor_copy(wo1[:], wo1_raw[:])   # absorb DMA wait off TT path
        ones8 = cpool.tile([128, 32], DT.float8e4)
        nc.vector.memset(ones8[:], 1.0)

        # psum accumulation groups per query-half
        S_ps = [ps.tile([32, 512], DT.float32, tag=f"S{h}", name=f"S{h}") for h in range(2)]
        C_ps = [ps.tile([32, 512], DT.float32, tag=f"C{h}", name=f"C{h}") for h in range(2)]

        # ---- matmul phase: stream mask tiles, accumulate ----
        for q in range(NQ):
            m0t = mpool.tile([128, QH], DT.uint16, tag="m0")
            nc.sync.dma_start(m0t[:], m0_d[:, 128 * q:128 * (q + 1)], transpose=True)
            m1t = mpool.tile([128, QH], DT.uint16, tag="m1")
            m1dma = nc.sync.dma_start(m1t[:], m1_d[:, 128 * q:128 * (q + 1)], transpose=True)
            m0r = m0t[:].bitcast(DT.float8e4).rearrange("p (i two) -> p i two", two=2)
            m1r = m1t[:].bitcast(DT.float8e4).rearrange("p (i two) -> p i two", two=2)
            for h in range(2):
                for par in range(2):
                    rhs0 = m0r[:, 512 * h:512 * (h + 1), par]
                    first = (q == 0 and par == 0)
                    last = (q == NQ - 1 and par == 1)
                    nc.tensor.matmul(S_ps[h][0:17, :], whi[:, q, par, :], rhs0,
                                     start=first, stop=False)
                    nc.tensor.matmul(S_ps[h][0:17, :], wlo[:, q, par, :], rhs0,
                                     start=False, stop=last)
                    rhs1 = m1r[:, 512 * h:512 * (h + 1), par]
                    last_mm = nc.tensor.matmul(C_ps[h][:], ones8[:], rhs1,
                                               start=first, stop=last)

        # x loads on Pool/SWDGE after all transpose DMAs (xbar-mode safety,
        # and they double as the xbar fence for the out-stores); only needed
        # in the post phase, so the delay hides under the matmul tail.

        # chain of tiny Pool DMAs, each absorbing exactly one sem for the
        # 1-wait-limited Pool out-stores: xbar serialization, then the two
        # x-load lanes
        xfence = cpool.tile([1, 3], DT.float32, name="xfence")
        f1 = nc.gpsimd.dma_start(xfence[0:1, 0:1], wo1_d[0:1, :])
        add_dep_helper(f1.ins, m1dma.ins, reason="xbar fence after last transpose")
        f2 = nc.gpsimd.dma_start(xfence[0:1, 1:2], wo1_d[0:1, :])
        add_dep_helper(f2.ins, xdmas[0].ins, reason="absorb x-load h0 lane")
        add_dep_helper(f2.ins, f1.ins, sync=False, reason="pool order")
        f3 = nc.gpsimd.dma_start(xfence[0:1, 2:3], wo1_d[0:1, :])
        add_dep_helper(f3.ins, xdmas[1].ins, reason="absorb x-load h1 lane")
        add_dep_helper(f3.ins, f2.ins, sync=False, reason="pool order")

        # ---- post phase per query-half ----
        for h in range(2):
            Ssb = wpool.tile([32, 512], DT.float32, tag="Ssb")
            nc.scalar.copy(Ssb[:], S_ps[h][:])
            Csb = wpool.tile([32, 512], DT.float32, tag="Csb")
            nc.scalar.copy(Csb[:], C_ps[h][:])

            TS = wpool.tile([32, 512], DT.float32, tag="TS")
            nc.vector.transpose(TS[:], Ssb[:])      # 16 in-place 32x32 blocks
            TC = wpool.tile([32, 512], DT.float32, tag="TC")
            nc.vector.transpose(TC[:], Csb[:])

            # att[128p, t, d] = S[d, 128t+p]; block (4t+m) of TS holds rows 32m..32m+32
            att = wpool.tile([128, 4, 32], DT.float32, tag="att")
            TSv = TS[:].rearrange("p (k d) -> p k d", d=32)     # [32, 16, 32]
            TCv = TC[:].rearrange("p (k d) -> p k d", d=32)
            for m in range(4):
                nc.vector.tensor_copy(att[32 * m:32 * m + 32, :, :], TSv[:, m::4, :])

            # scale chain on [128, 4]: c0 = att[:, :, 16]; w = r/(1-r), r=1/(1+c0)
            denom = wpool.tile([128, 4], DT.float32, tag="denom")
            nc.vector.tensor_scalar_add(denom[:], att[:, :, 16], 1.0)
            r_t = wpool.tile([128, 4], DT.float32, tag="r_t")
            nc.vector.reciprocal(r_t[:], denom[:])
            omr = wpool.tile([128, 4], DT.float32, tag="omr")
            nc.vector.tensor_scalar(omr[:], r_t[:], -1.0, 1.0, AL.mult, AL.add)
            nc.vector.tensor_scalar_max(omr[:], omr[:], 1e-9)
            romr = wpool.tile([128, 4], DT.float32, tag="romr")
            nc.vector.reciprocal(romr[:], omr[:])
            wcol = wpool.tile([128, 4], DT.float32, tag="wcol")
            nc.vector.tensor_tensor(wcol[:], r_t[:], romr[:], AL.mult)

            # output staging: x rows stream straight into the out tile
            ot = ots[h]
            lab = wpool.tile([1, 1], DT.float32, tag="lab", name=f"lab{h}")
            abs_cp = nc.vector.tensor_copy(lab[:], ot[0:1, 0, 0:1])

            atts = wpool.tile([128, 4, 16], DT.float32, tag="atts")
            for t in range(4):
                nc.vector.scalar_tensor_tensor(
                    atts[:, t, :], att[:, t, 0:16], wcol[:, t:t + 1],
                    att[:, t, 0:16], AL.mult, AL.bypass)

            # ch16:32 = attended
            cp1632 = nc.vector.tensor_copy(ot[:, :, 16:32], atts[:])
            add_dep_helper(cp1632.ins, abs_cp.ins, sync=False, reason="after lane absorb")
            # ch32 = c1 * W_o1 (gather TC blocks to full partitions first)
            c1col = wpool.tile([128, 4], DT.float32, tag="c1col")
            for m in range(4):
                nc.vector.tensor_copy(c1col[32 * m:32 * m + 32, :], TCv[:, m::4, 0])
            cstt = nc.vector.scalar_tensor_tensor(
                ot[:, :, 32:33].rearrange("p t one -> p (t one)"),
                c1col[:], wo1[:], c1col[:], AL.mult, AL.bypass)
            add_dep_helper(cstt.ins, abs_cp.ins, sync=False, reason="after lane absorb")
            # MLP: a = ch0:16, b = atts
            for alu, lo in ((AL.add, 48), (AL.mult, 64), (AL.is_lt, 80)):
                mlp = nc.vector.tensor_tensor(ot[:, :, lo:lo + 16], atts[:],
                                              ot[:, :, 0:16], alu)
                add_dep_helper(mlp.ins, abs_cp.ins, sync=False, reason="after lane absorb")

            nc.gpsimd.dma_start(o_view[:, 4 * h:4 * (h + 1), :], ot[:])

    return nc


_cached = {}


def _prepare_in_maps(x, mask0, mask1, W_o0, W_o1):
    x = np.asarray(x, dtype=np.float32)
    m0u8 = np.asarray(mask0).astype(np.uint8, copy=False)
    m1u8 = np.asarray(mask1).astype(np.uint8, copy=False)
    W_o0 = np.asarray(W_o0, dtype=np.float32)
    W_o1 = np.asarray(W_o1, dtype=np.float32)

    # u = values through the head-0 output projection; hi/lo split, x512
    in_maps = []
    for c in range(8):
        b, h = divmod(c, 2)
        u = x[b, :, 0:16] @ W_o0.T                      # (S, 16) f32
        u_hi = u.astype(ml_dtypes.bfloat16)
        u_lo = (u - u_hi.astype(np.float32)).astype(ml_dtypes.bfloat16)
        whi = np.zeros((128, NQ, 2, 17), dtype=ml_dtypes.bfloat16)
        wlo = np.zeros((128, NQ, 2, 17), dtype=ml_dtypes.bfloat16)
        for q in range(NQ):
            blk_hi = u_hi[256 * q:256 * (q + 1)]        # (256, 16)
            blk_lo = u_lo[256 * q:256 * (q + 1)]
            for par in range(2):
                whi[:, q, par, :16] = (blk_hi[par::2].astype(np.float32) * 512.0
                                       ).astype(ml_dtypes.bfloat16)
                wlo[:, q, par, :16] = (blk_lo[par::2].astype(np.float32) * 512.0
                                       ).astype(ml_dtypes.bfloat16)
            whi[:, q, :, 16] = 512.0                    # ones column -> c0 exactly
        sl = slice(QH * h, QH * (h + 1))
        in_maps.append({
            "m0": np.ascontiguousarray(m0u8[b, sl, :]).view(np.uint16),
            "m1": np.ascontiguousarray(m1u8[b, sl, :]).view(np.uint16),
            "xq": np.ascontiguousarray(x[b, sl, :]),
            "whi": whi,
            "wlo": wlo,
            "wo1": np.full((128, 1), 512.0 * float(W_o1[0, 0]), np.float32),
        })
    return in_maps


def kernel(x, mask0, mask1, W_o0, W_o1):
    if "nc" not in _cached:
        _cached["nc"] = _build_program()
    nc = _cached["nc"]
    in_maps = _prepare_in_maps(x, mask0, mask1, W_o0, W_o1)
    res = run_bass_kernel_spmd(nc, in_maps, list(range(8)))
    _cached["last_results"] = res
    out = np.empty((B, S, D), np.float32)
    for c in range(8):
        b, h = divmod(c, 2)
        out[b, QH * h:QH * (h + 1), :] = res.results[c]["out"]
    return out


# revision 21
# speedup vs baseline: 27343.1400x; 2351.9085x over previous
"""Trainium2 Bass kernel for the sparse-attention CompiledTransformerLayer.

Math (derived from the reference):
  c0 = rowsum(mask0); attended = (mask0 @ x[:,:,0:16]) * r/(1-r), r = 1/(1+c0)
  out ch16:32 = attended @ W_o0.T
  out ch32    = c1 * W_o1[0,0], c1 = rowsum(mask1)
  out ch48:64 = a + b; 64:80 = a*b; 80:96 = (a > b), a = x ch0:16, b = ch16:32
  all other channels pass through from x.

Sharding: 8 cores = 4 batches x 2 query-halves (1024 queries each).

Tricks:
  - bool masks are DMA-transposed as uint16 byte-pairs (HWDGE xbar, 2-byte dtype),
    then fed to the PE matmul directly as float8e4: byte 0x01 is the fp8
    denormal 2^-9, so results are exactly scaled by 2^-9 (weights pre-scaled
    by 512 to compensate).
  - value weights are (x[:,:,0:16] @ W_o0.T) split hi+lo in bf16 for ~f32
    matmul precision; an extra ones*512 column yields c0 in the same psum.
  - rowsum(mask1) via an all-ones fp8 stationary matmul (exact).
"""
import sys
sys.path.insert(0, "/opt/trn_rl_repo")
import numpy as np
import ml_dtypes

import concourse.bass as bass
import concourse.mybir as mybir
from concourse import tile
from concourse.bass_utils import run_bass_kernel_spmd
from concourse.vector_clock import ScopedClock, VectorClock
from concourse.tile import add_dep_helper

B, S, D = 4, 2048, 128
QH = S // 2              # queries per core
NQ = 8                   # j2 blocks of 128 (each covers 256 keys)
DT = mybir.dt
AL = mybir.AluOpType

# walrus codegen rejects instructions with many sem waits; the Tile tail
# drain accumulates one wait per touched proc. Emit one single-wait drain
# per proc instead.
def _patched_dab(self, tick_clock, wait_clock):
    ticks = list(tick_clock.global_clock)
    for i, t in enumerate(ticks):
        if t <= 0:
            continue
        part = [t if j == i else 0 for j, t in enumerate(ticks)]
        d = self.nc.sync.drain()
        wait_clock.add_sem_waits(d.ins, ScopedClock({None: VectorClock(part)}))
    self.nc.sync.drain()
    self.nc.all_engine_barrier()
    popped = self.nc._tile_sem_poison_stack.pop()
    assert popped is self._sem_poison
    self.nc.clear_and_free_semaphores(list(self.sems.allocated().values()))
    self.nc.all_engine_barrier()
tile.TileContext._drain_and_barrier = _patched_dab


def _build_program():
    nc = bass.Bass()
    m0_d = nc.declare_dram_parameter("m0", [QH, S // 2], DT.uint16, isOutput=False)
    m1_d = nc.declare_dram_parameter("m1", [QH, S // 2], DT.uint16, isOutput=False)
    x_d = nc.declare_dram_parameter("xq", [QH, D], DT.float32, isOutput=False)
    whi_d = nc.declare_dram_parameter("whi", [128, NQ, 2, 17], DT.bfloat16, isOutput=False)
    wlo_d = nc.declare_dram_parameter("wlo", [128, NQ, 2, 17], DT.bfloat16, isOutput=False)
    wo1_d = nc.declare_dram_parameter("wo1", [128, 1], DT.float32, isOutput=False)
    out_d = nc.declare_dram_parameter("out", [QH, D], DT.float32, isOutput=True)

    x_view = None  # set below
    with tile.TileContext(nc) as tc, \
         tc.tile_pool(name="const", bufs=1) as cpool, \
         tc.tile_pool(name="masks", bufs=8) as mpool, \
         tc.tile_pool(name="work", bufs=2) as wpool, \
         tc.tile_pool(name="ps", bufs=1, space="PSUM") as ps:

        x_view = x_d[:].rearrange("(t p) c -> p t c", p=128)    # [128, 8, 128]
        o_view = out_d[:].rearrange("(t p) c -> p t c", p=128)

        # x loads first: zero-wait DMAs, and they precede every transpose so
        # the xbar-mode serialization never lands on them
        ots = []
        xdmas = []
        for h in range(2):
            ot = wpool.tile([128, 4, D], DT.float32, tag=f"ot{h}", name=f"ot{h}")
            xdmas.append(nc.sync.dma_start(ot[:], x_view[:, 4 * h:4 * (h + 1), :]))
            ots.append(ot)

        whi = cpool.tile([128, NQ, 2, 17], DT.bfloat16)
        wlo = cpool.tile([128, NQ, 2, 17], DT.bfloat16)
        nc.sync.dma_start(whi[:], whi_d[:])
        nc.sync.dma_start(wlo[:], wlo_d[:])
        wo1_raw = cpool.tile([128, 1], DT.float32)
        nc.sync.dma_start(wo1_raw[:], wo1_d[:])
        wo1 = cpool.tile([128, 1], DT.float32)
        nc.vector.tensor_copy(wo1[:], wo1_raw[:])   # absorb DMA wait off TT path
        ones8 = cpool.tile([128, 32], DT.float8e4)
        nc.vector.memset(ones8[:], 1.0)

        # psum accumulation groups per query-half
        S_ps = [ps.tile([32, 512], DT.float32, tag=f"S{h}", name=f"S{h}") for h in range(2)]
        C_ps = [ps.tile([32, 512], DT.float32, tag=f"C{h}", name=f"C{h}") for h in range(2)]

        # ---- matmul phase: stream mask tiles (all resident), h outer so the
        # h=0 post phase overlaps the h=1 matmuls ----
        m0rs, m1rs = [], []
        for q in range(NQ):
            m0t = mpool.tile([128, QH], DT.uint16, tag="m0")
            m0dma = nc.sync.dma_start(m0t[:], m0_d[:, 128 * q:128 * (q + 1)], transpose=True)
            m1t = mpool.tile([128, QH], DT.uint16, tag="m1")
            m1dma = nc.sync.dma_start(m1t[:], m1_d[:, 128 * q:128 * (q + 1)], transpose=True)
            m0rs.append(m0t[:].bitcast(DT.float8e4).rearrange("p (i two) -> p i two", two=2))
            m1rs.append(m1t[:].bitcast(DT.float8e4).rearrange("p (i two) -> p i two", two=2))
        for h in range(2):
            for q in range(NQ):
                for par in range(2):
                    rhs0 = m0rs[q][:, 512 * h:512 * (h + 1), par]
                    first = (q == 0 and par == 0)
                    last = (q == NQ - 1 and par == 1)
                    nc.tensor.matmul(S_ps[h][0:17, :], whi[:, q, par, :], rhs0,
                                     start=first, stop=False)
                    nc.tensor.matmul(S_ps[h][0:17, :], wlo[:, q, par, :], rhs0,
                                     start=False, stop=last)
                    rhs1 = m1rs[q][:, 512 * h:512 * (h + 1), par]
                    last_mm = nc.tensor.matmul(C_ps[h][:], ones8[:], rhs1,
                                               start=first, stop=last)

        # x loads on Pool/SWDGE after all transpose DMAs (xbar-mode safety,
        # and they double as the xbar fence for the out-stores); only needed
        # in the post phase, so the delay hides under the matmul tail.

        # chain of tiny Pool DMAs, each absorbing exactly one sem for the
        # 1-wait-limited Pool out-stores: xbar serialization, then the two
        # x-load lanes
        xfence = cpool.tile([1, 4], DT.float32, name="xfence")
        f1 = nc.gpsimd.dma_start(xfence[0:1, 0:1], wo1_d[0:1, :])
        add_dep_helper(f1.ins, m1dma.ins, reason="xbar fence after last m1 transpose")
        f1b = nc.gpsimd.dma_start(xfence[0:1, 3:4], wo1_d[0:1, :])
        add_dep_helper(f1b.ins, m0dma.ins, reason="xbar fence after last m0 transpose")
        add_dep_helper(f1b.ins, f1.ins, sync=False, reason="pool order")
        f2 = nc.gpsimd.dma_start(xfence[0:1, 1:2], wo1_d[0:1, :])
        add_dep_helper(f2.ins, xdmas[0].ins, reason="absorb x-load h0 lane")
        add_dep_helper(f2.ins, f1.ins, sync=False, reason="pool order")
        f3 = nc.gpsimd.dma_start(xfence[0:1, 2:3], wo1_d[0:1, :])
        add_dep_helper(f3.ins, xdmas[1].ins, reason="absorb x-load h1 lane")
        add_dep_helper(f3.ins, f2.ins, sync=False, reason="pool order")

        # ---- post phase per query-half ----
        for h in range(2):
            Ssb = wpool.tile([32, 512], DT.float32, tag="Ssb")
            nc.scalar.copy(Ssb[:], S_ps[h][:])
            Csb = wpool.tile([32, 512], DT.float32, tag="Csb")
            nc.scalar.copy(Csb[:], C_ps[h][:])

            TS = wpool.tile([32, 512], DT.float32, tag="TS")
            nc.vector.transpose(TS[:], Ssb[:])      # 16 in-place 32x32 blocks
            TC = wpool.tile([32, 512], DT.float32, tag="TC")
            nc.vector.transpose(TC[:], Csb[:])

            # att[128p, t, d] = S[d, 128t+p]; block (4t+m) of TS holds rows 32m..32m+32
            att = wpool.tile([128, 4, 32], DT.float32, tag="att")
            TSv = TS[:].rearrange("p (k d) -> p k d", d=32)     # [32, 16, 32]
            TCv = TC[:].rearrange("p (k d) -> p k d", d=32)
            for m in range(4):
                nc.vector.tensor_copy(att[32 * m:32 * m + 32, :, :], TSv[:, m::4, :])

            # scale chain on [128, 4]: c0 = att[:, :, 16]; w = r/(1-r), r=1/(1+c0)
            denom = wpool.tile([128, 4], DT.float32, tag="denom")
            nc.vector.tensor_scalar_add(denom[:], att[:, :, 16], 1.0)
            r_t = wpool.tile([128, 4], DT.float32, tag="r_t")
            nc.vector.reciprocal(r_t[:], denom[:])
            omr = wpool.tile([128, 4], DT.float32, tag="omr")
            nc.vector.tensor_scalar(omr[:], r_t[:], -1.0, 1.0, AL.mult, AL.add)
            nc.vector.tensor_scalar_max(omr[:], omr[:], 1e-9)
            romr = wpool.tile([128, 4], DT.float32, tag="romr")
            nc.vector.reciprocal(romr[:], omr[:])
            wcol = wpool.tile([128, 4], DT.float32, tag="wcol")
            nc.vector.tensor_tensor(wcol[:], r_t[:], romr[:], AL.mult)

            # output staging: x rows stream straight into the out tile
            ot = ots[h]
            lab = wpool.tile([1, 1], DT.float32, tag="lab", name=f"lab{h}")
            abs_cp = nc.vector.tensor_copy(lab[:], ot[0:1, 0, 0:1])

            atts = wpool.tile([128, 4, 16], DT.float32, tag="atts")
            for t in range(4):
                nc.vector.scalar_tensor_tensor(
                    atts[:, t, :], att[:, t, 0:16], wcol[:, t:t + 1],
                    att[:, t, 0:16], AL.mult, AL.bypass)

            # ch16:32 = attended
            cp1632 = nc.vector.tensor_copy(ot[:, :, 16:32], atts[:])
            add_dep_helper(cp1632.ins, abs_cp.ins, sync=False, reason="after lane absorb")
            # ch32 = c1 * W_o1 (gather TC blocks to full partitions first)
            c1col = wpool.tile([128, 4], DT.float32, tag="c1col")
            for m in range(4):
                nc.vector.tensor_copy(c1col[32 * m:32 * m + 32, :], TCv[:, m::4, 0])
            cstt = nc.vector.scalar_tensor_tensor(
                ot[:, :, 32:33].rearrange("p t one -> p (t one)"),
                c1col[:], wo1[:], c1col[:], AL.mult, AL.bypass)
            add_dep_helper(cstt.ins, abs_cp.ins, sync=False, reason="after lane absorb")
            # MLP: a = ch0:16, b = atts
            for alu, lo in ((AL.add, 48), (AL.mult, 64), (AL.is_lt, 80)):
                mlp = nc.vector.tensor_tensor(ot[:, :, lo:lo + 16], atts[:],
                                              ot[:, :, 0:16], alu)
                add_dep_helper(mlp.ins, abs_cp.ins, sync=False, reason="after lane absorb")

            nc.gpsimd.dma_start(o_view[:, 4 * h:4 * (h + 1), :], ot[:])

    return nc


_cached = {}


def _prepare_in_maps(x, mask0, mask1, W_o0, W_o1):
    x = np.asarray(x, dtype=np.float32)
    m0u8 = np.asarray(mask0).astype(np.uint8, copy=False)
    m1u8 = np.asarray(mask1).astype(np.uint8, copy=False)
    W_o0 = np.asarray(W_o0, dtype=np.float32)
    W_o1 = np.asarray(W_o1, dtype=np.float32)

    # u = values through the head-0 output projection; hi/lo split, x512
    in_maps = []
    for c in range(8):
        b, h = divmod(c, 2)
        u = x[b, :, 0:16] @ W_o0.T                      # (S, 16) f32
        u_hi = u.astype(ml_dtypes.bfloat16)
        u_lo = (u - u_hi.astype(np.float32)).astype(ml_dtypes.bfloat16)
        whi = np.zeros((128, NQ, 2, 17), dtype=ml_dtypes.bfloat16)
        wlo = np.zeros((128, NQ, 2, 17), dtype=ml_dtypes.bfloat16)
        for q in range(NQ):
            blk_hi = u_hi[256 * q:256 * (q + 1)]        # (256, 16)
            blk_lo = u_lo[256 * q:256 * (q + 1)]
            for par in range(2):
                whi[:, q, par, :16] = (blk_hi[par::2].astype(np.float32) * 512.0
                                       ).astype(ml_dtypes.bfloat16)
                wlo[:, q, par, :16] = (blk_lo[par::2].astype(np.float32) * 512.0
                                       ).astype(ml_dtypes.bfloat16)
            whi[:, q, :, 16] = 512.0                    # ones column -> c0 exactly
        sl = slice(QH * h, QH * (h + 1))
        in_maps.append({
            "m0": np.ascontiguousarray(m0u8[b, sl, :]).view(np.uint16),
            "m1": np.ascontiguousarray(m1u8[b, sl, :]).view(np.uint16),
            "xq": np.ascontiguousarray(x[b, sl, :]),
            "whi": whi,
            "wlo": wlo,
            "wo1": np.full((128, 1), 512.0 * float(W_o1[0, 0]), np.float32),
        })
    return in_maps


def kernel(x, mask0, mask1, W_o0, W_o1):
    if "nc" not in _cached:
        _cached["nc"] = _build_program()
    nc = _cached["nc"]
    in_maps = _prepare_in_maps(x, mask0, mask1, W_o0, W_o1)
    res = run_bass_kernel_spmd(nc, in_maps, list(range(8)))
    _cached["last_results"] = res
    out = np.empty((B, S, D), np.float32)
    for c in range(8):
        b, h = divmod(c, 2)
        out[b, QH * h:QH * (h + 1), :] = res.results[c]["out"]
    return out
